# revision 1
# baseline (speedup 1.0000x reference)
"""Trainium2 Bass kernel for the ragged text-CNN problem.

Math: for tokens x[t,b] with embedding tables E,U [V,D] and conv
weights w [H, 2D, 2] (kernel size 2 over time):

    conv[b,h,t] = w0_h . e_{t,b} + w1_h . e_{t+1,b} + cb_h
    scores      = (max over valid t of conv) @ out_w.T + out_b

where e = concat(E[tok], U[tok]).  We precompute a fused table

    T[v, 0:64]   = concat(E[v],U[v]) . w0_h        (the "A" half)
    T[v, 64:128] = concat(E[v],U[v]) . w1_h        (the "B" half)

so conv[b,h,t] = T[tok_t, h] + T[tok_{t+1}, 64+h].  The ragged mask is
free: PAD (=1) appears exactly at positions t >= len, so forcing
T[1, 0:64] = -1e30 makes every masked conv position -1e30.

Distribution over 8 cores: phase A (table build) is vocab-sharded
(V/8 rows each) and exchanged with an AllGather; phase B (gather +
conv + masked max + linear head) is batch-sharded (B/8 sentences).
"""

import numpy as np

try:
    import concourse.bass as bass
except ImportError:  # harness runs from a bare directory
    import sys

    sys.path.insert(0, "/opt/trn_rl_repo")
    import concourse.bass as bass

import concourse.bass_isa as bass_isa
import concourse.mybir as mybir
from concourse.bacc import Bacc
import concourse.tile as tile
from concourse.bass_utils import run_bass_kernel_spmd
from concourse.masks import make_identity

V, D, H, S, B = 50000, 300, 64, 512, 256
NCORES = 8
VS = V // NCORES        # vocab rows per core (6250)
BS = B // NCORES        # sentences per core (32)
F = 2 * H               # fused feature width (128)
NEG = -1.0e30
P = 128

F32 = mybir.dt.float32
BF16 = mybir.dt.bfloat16
I32 = mybir.dt.int32


def build_nc(v=V, d=D, h=H, s=S, bs=BS, ncores=NCORES, mm_bf16=True, debug_probe=False, stop_after=None):
    """Build the per-core SPMD Bass program (identical on all cores)."""
    vs = v // ncores
    f = 2 * h
    kb = s // P
    assert s % P == 0 and v % ncores == 0
    fh = h * bs
    n_vt = (vs + P - 1) // P
    chunks = []  # (src_is_u, col0, width) over the 2D concat dim
    for base in range(0, d, P):
        chunks.append((False, base, min(P, d - base)))
    for base in range(0, d, P):
        chunks.append((True, base, min(P, d - base)))
    nch = len(chunks)
    mm_dt = BF16 if mm_bf16 else F32
    t_dt = mm_dt

    nc = Bacc()
    e_sh = nc.dram_tensor("e_shard", [vs, d], F32, kind="ExternalInput")
    u_sh = nc.dram_tensor("u_shard", [vs, d], F32, kind="ExternalInput")
    sent = nc.dram_tensor("sent", [s, bs], I32, kind="ExternalInput")
    sent2 = nc.dram_tensor("sent2", [s, bs], I32, kind="ExternalInput")
    convw = nc.dram_tensor("conv_w", [h, d * 2, 2], F32, kind="ExternalInput")
    convb = nc.dram_tensor("conv_b", [1, h], F32, kind="ExternalInput")
    outw = nc.dram_tensor("out_w", [2, h], F32, kind="ExternalInput")
    outb = nc.dram_tensor("out_b", [1, 2], F32, kind="ExternalInput")
    patch = nc.dram_tensor("patch", [2, f], F32, kind="ExternalInput")
    t_shard = nc.dram_tensor("t_shard", [vs, f], t_dt)
    t_full = nc.dram_tensor("t_full", [v + 1, f], t_dt, addr_space="Shared")
    scores = nc.dram_tensor("scores", [bs, 2], F32, kind="ExternalOutput")
    if debug_probe:
        tsh_out = nc.dram_tensor("tsh_out", [vs, f], F32, kind="ExternalOutput")
        tf_out = nc.dram_tensor("tf_out", [v, f], F32, kind="ExternalOutput")
        ga_out = nc.dram_tensor("ga_out", [P, kb * bs * h], F32, kind="ExternalOutput")
        gb_out = nc.dram_tensor("gb_out", [P, kb * bs * h], F32, kind="ExternalOutput")

    with tile.TileContext(nc) as tc:
        with tc.tile_pool(name="const", bufs=1) as cpool:
            ident = cpool.tile([P, P], F32, tag="identf")
            make_identity(nc, ident[:])
            identb = cpool.tile([P, P], mm_dt, tag="identb")
            if mm_bf16:
                make_identity(nc, identb[:])

            # ---- W2 prep: w2c[ci][dd, k*h + hh] = conv_w[hh, c0+dd, k]
            cw_sb = cpool.tile([h, d * 2 * 2], F32, tag="cw")
            nc.sync.dma_start(
                cw_sb[:], convw[:, :, :].rearrange("a b c -> a (b c)")
            )
            cw_v = cw_sb[:].rearrange("a (b c) -> a b c", c=2)
            w2cs = []
            with tc.tile_pool(name="w2psum", bufs=2, space="PSUM") as wpp:
                for ci, (_, c0, dc) in enumerate(chunks):
                    w2c = cpool.tile([P, f], mm_dt, tag=f"w2_{ci}")
                    w2cs.append(w2c)
                    cglob = c0 + (d if chunks[ci][0] else 0)
                    for k in range(2):
                        tp = wpp.tile([P, h], F32, tag="wtp")
                        nc.tensor.transpose(
                            tp[:dc, :h],
                            cw_v[:, cglob : cglob + dc, k],
                            ident[:h, :h],
                        )
                        nc.vector.tensor_copy(
                            w2c[:dc, k * h : (k + 1) * h], tp[:dc, :h]
                        )

            patch_sb = cpool.tile([2, f], t_dt, tag="patch")
            nc.gpsimd.dma_start(patch_sb[:], patch[:, :])

            # ---- Phase A: T_shard tiles
            with (
                tc.tile_pool(name="pa", bufs=3) as papool,
                tc.tile_pool(name="pa_ps", bufs=3, space="PSUM") as papsum,
                tc.tile_pool(name="pa_acc", bufs=2, space="PSUM") as paacc,
            ):
                for ti in range(n_vt):
                    r0 = ti * P
                    nr = min(P, vs - r0)
                    e_t = papool.tile([P, d], mm_dt, tag="e_t")
                    u_t = papool.tile([P, d], mm_dt, tag="u_t")
                    if mm_bf16:
                        nc.gpsimd.dma_start(e_t[:nr], e_sh[r0 : r0 + nr, :])
                        nc.gpsimd.dma_start(u_t[:nr], u_sh[r0 : r0 + nr, :])
                    else:
                        nc.sync.dma_start(e_t[:nr], e_sh[r0 : r0 + nr, :])
                        nc.sync.dma_start(u_t[:nr], u_sh[r0 : r0 + nr, :])
                    ets = papool.tile([P, nch * P], mm_dt, tag="ets")
                    for ci, (is_u, c0, dc) in enumerate(chunks):
                        src = u_t if is_u else e_t
                        tp = papsum.tile([P, P], mm_dt, tag="tp")
                        nc.tensor.transpose(
                            tp[:dc, :nr],
                            src[:nr, c0 : c0 + dc],
                            identb[:nr, :nr] if mm_bf16 else ident[:nr, :nr],
                        )
                        nc.any.tensor_copy(
                            ets[:dc, ci * P : ci * P + nr], tp[:dc, :nr]
                        )
                    acc = paacc.tile([P, f], F32, tag="acc")
                    for ci, (is_u, c0, dc) in enumerate(chunks):
                        nc.tensor.matmul(
                            acc[:nr, :],
                            lhsT=ets[:dc, ci * P : ci * P + nr],
                            rhs=w2cs[ci][:dc, :],
                            start=(ci == 0),
                            stop=(ci == nch - 1),
                        )
                    t_sb = papool.tile([P, f], t_dt, tag="t_sb")
                    nc.any.tensor_copy(t_sb[:nr], acc[:nr])
                    if ti == 0:
                        # core 0's patch is -1e30 on the A half; others zero
                        nc.vector.tensor_add(
                            t_sb[0:2, :], t_sb[0:2, :], patch_sb[0:2, :]
                        )
                    nc.sync.dma_start(t_shard[r0 : r0 + nr, :], t_sb[:nr])

            # ---- exchange shards
            nc.gpsimd.collective_compute(
                "AllGather",
                mybir.AluOpType.bypass,
                replica_groups=[list(range(ncores))],
                ins=[t_shard[:, :]],
                outs=[t_full[0:v, :]],
            )

            if debug_probe:
                nc.sync.dma_start(tsh_out[:, :], t_shard[:, :])
                nc.sync.dma_start(tf_out[:, :], t_full[:, :])

            neg_sb = cpool.tile([1, f], t_dt, tag="negrow")
            nc.vector.memset(neg_sb[:], NEG)
            nc.sync.dma_start(t_full[v : v + 1, :], neg_sb[:])

            # ---- Phase B: gather + conv + masked max + head
            with (
                tc.tile_pool(name="pb", bufs=1) as pbpool,
                tc.tile_pool(name="pbh", bufs=1) as hpool,
                tc.tile_pool(name="pb_ps", bufs=4, space="PSUM") as pbpsum,
            ):
                # --- token gather via dma_gather (int16 idx; split table)
                # idx order i = b*s + t  ->  out[p=i%128, j=i//128] with
                # j = b*kb + k, t = k*128 + p.
                nidx = s * bs
                nw = nidx // 16
                nj = nidx // P
                split = 32768 if v > 32768 else (v + 1) // 2
                # wrapped idx layout: idx i=b*s+t at (partition i%16, col
                # i//16) = (t%16, b*(s//16)+t//16); the queue-q gather reads
                # partitions [32q, 32q+32), so replicate the wrap into each
                # group a call needs: swa groups 0-3 (queues 0,1), swb
                # groups 4-7 (queues 2,3).
                def load_wrapped(dst, src_dram, groups):
                    for g in groups:
                        nc.sync.dma_start(
                            dst[16 * g : 16 * (g + 1), :],
                            bass.AP(
                                src_dram,
                                0,
                                [[bs, 16], [1, bs], [16 * bs, s // 16]],
                            ),
                        )

                swa = pbpool.tile([P, nw], I32, tag="swa")
                swb = pbpool.tile([P, nw], I32, tag="swb")
                load_wrapped(swa, sent, range(0, 2))
                load_wrapped(swb, sent2, range(0, 2))
                ilo_a = pbpool.tile([P, nw], mybir.dt.int16, tag="ilo_a")
                ihi_a = pbpool.tile([P, nw], mybir.dt.int16, tag="ihi_a")
                ilo_b = pbpool.tile([P, nw], mybir.dt.int16, tag="ilo_b")
                ihi_b = pbpool.tile([P, nw], mybir.dt.int16, tag="ihi_b")
                # queue-0 cores read idxs from partitions [0,16) and [16,32)
                # ilo = tok < split ? tok : 0        (row 0 = -1e30)
                # ihi = tok >= split ? tok-split : v-split   (row v = -1e30)
                for t in (ilo_a, ihi_a, ilo_b, ihi_b):
                    nc.vector.memset(t[:], 0)
                c2 = pbpool.tile([P, nw], I32, tag="c2")
                c1 = pbpool.tile([P, nw], I32, tag="c1")
                d2 = pbpool.tile([P, nw], I32, tag="d2")
                for sw, ilo, ihi in ((swa, ilo_a, ihi_a), (swb, ilo_b, ihi_b)):
                    nc.vector.tensor_scalar(
                        c2[0:32], sw[0:32], split, None, mybir.AluOpType.is_ge
                    )
                    nc.vector.tensor_scalar(
                        c1[0:32], sw[0:32], split, None, mybir.AluOpType.is_lt
                    )
                    nc.vector.tensor_tensor(
                        ilo[0:32], sw[0:32], c1[0:32], op=mybir.AluOpType.mult
                    )
                    nc.vector.tensor_scalar(
                        d2[0:32], sw[0:32], v, None, mybir.AluOpType.subtract
                    )
                    nc.vector.tensor_tensor(
                        d2[0:32], d2[0:32], c2[0:32], op=mybir.AluOpType.mult
                    )
                    nc.vector.tensor_scalar(
                        ihi[0:32], d2[0:32], v - split, None, mybir.AluOpType.add
                    )
                ga = pbpool.tile([P, nj * f], t_dt, tag="ga")
                gah = pbpool.tile([P, nj * f], t_dt, tag="gah")
                gb = pbpool.tile([P, nj * f], t_dt, tag="gb")
                gbh = pbpool.tile([P, nj * f], t_dt, tag="gbh")
                gathers = [
                    (ga, t_full[0 : split, :], ilo_a, 0),
                    (gah, t_full[split : v + 1, :], ihi_a, 0),
                    (gb, t_full[0 : split, :], ilo_b, 0),
                    (gbh, t_full[split : v + 1, :], ihi_b, 0),
                ]
                # ring carveout holds 2048 descs/direction; one call may
                # carry at most ~16k idxs (descs = nidx/16 + 1), so chunk.
                max_chunk = 8192
                chunks_i = []
                i0 = 0
                while i0 < nidx:
                    cn = min(max_chunk, nidx - i0)
                    chunks_i.append((i0, cn))
                    i0 += cn
                for out_t, in_ap, idx_t, q in gathers:
                    ov = out_t[:].rearrange("p (j c) -> p j c", c=f)
                    for i0, cn in chunks_i:
                        nc.gpsimd.dma_gather(
                            out_ap=ov[:, i0 // P : (i0 + cn) // P, :],
                            in_ap=in_ap,
                            idxs_ap=idx_t[:, i0 // 16 : (i0 + cn) // 16],
                            num_idxs=cn,
                            num_idxs_reg=cn,
                            elem_size=f,
                            elem_step=f,
                            queue_num=q,
                            single_packet=False,
                        )
                # merge: wrong-table entries are -1e30, so max picks
                # the real row
                nc.any.tensor_max(ga[:], ga[:], gah[:])
                nc.any.tensor_max(gb[:], gb[:], gbh[:])
                # conv[p, (b, k, c)] = ga.Ahalf + gb.Bhalf
                conv = pbpool.tile([P, nj * h], F32, tag="conv")
                gav = ga[:].rearrange("p (j c) -> p j c", c=f)
                gbv = gb[:].rearrange("p (j c) -> p j c", c=f)
                nc.any.tensor_add(
                    conv[:].rearrange("p (j c) -> p j c", c=h),
                    gav[:, :, 0:h],
                    gbv[:, :, h:f],
                )
                c4 = conv[:].rearrange("p (b k c) -> p b k c", b=bs, k=kb)
                # max over k blocks -> m [p, (b, h)]
                if kb > 1:
                    m = hpool.tile([P, fh], F32, tag="m")
                    nc.any.tensor_max(m[:], c4[:, :, 0, :], c4[:, :, 1, :])
                    for k in range(2, kb):
                        nc.any.tensor_max(m[:], m[:], c4[:, :, k, :])
                    m_ap = m[:]
                else:
                    m_ap = c4[:, :, 0, :]
                # per-sentence: transpose [128 tok, 64 feat] -> [64, 128] and
                # reduce over the 128 tokens, into pooled_t[:, b]
                pooled_t = pbpool.tile([h + 1, bs], F32, tag="pooled_t")
                nc.vector.memset(pooled_t[h : h + 1, :], 1.0)
                for b in range(bs):
                    mt = pbpsum.tile([h, P], F32, tag="mt")
                    msl = (
                        m[:, b * h : (b + 1) * h]
                        if kb > 1
                        else c4[:, b, 0, :]
                    )
                    nc.tensor.transpose(mt[:, :], msl, ident[:, :])
                    nc.vector.reduce_max(
                        pooled_t[0:h, b : b + 1],
                        mt[:, :],
                        axis=mybir.AxisListType.X,
                    )
                cb_t = pbpool.tile([h, 1], F32, tag="cb_t")
                nc.sync.dma_start(cb_t[:, :], convb[:, :].rearrange("o c -> c o"))
                nc.vector.tensor_scalar_add(
                    pooled_t[0:h, :], pooled_t[0:h, :], cb_t[:, :]
                )
                ow_t = pbpool.tile([h + 1, 2], F32, tag="ow_t")
                nc.sync.dma_start(ow_t[0:h, :], outw[:, :].rearrange("a c -> c a"))
                nc.sync.dma_start(ow_t[h : h + 1, :], outb[:, :])
                sc_ps = pbpsum.tile([bs, 2], F32, tag="sc")
                nc.tensor.matmul(
                    sc_ps[:, :],
                    lhsT=pooled_t[:, :],
                    rhs=ow_t[:, :],
                    start=True,
                    stop=True,
                )
                sc_sb = pbpool.tile([bs, 2], F32, tag="sc_sb")
                nc.vector.tensor_copy(sc_sb[:], sc_ps[:])
                nc.sync.dma_start(scores[:, :], sc_sb[:])

    nc.finalize()
    return nc


_NC_CACHE = {}


def _get_nc():
    if "nc" not in _NC_CACHE:
        _NC_CACHE["nc"] = build_nc()
    return _NC_CACHE["nc"]


def make_in_maps(sentences, E, U, conv_w, conv_b, out_w, out_b,
                 v=V, h=H, ncores=NCORES):
    vs = v // ncores
    bs = sentences.shape[1] // ncores
    f = 2 * h
    sent_shift = np.concatenate(
        [sentences[1:], np.zeros((1, sentences.shape[1]), np.int32)], axis=0
    )
    in_maps = []
    for c in range(ncores):
        pt = np.zeros((2, f), np.float32)
        if c == 0:
            pt[0, :] = NEG
            pt[1, :h] = NEG
        in_maps.append(
            {
                "e_shard": np.ascontiguousarray(E[c * vs : (c + 1) * vs]),
                "u_shard": np.ascontiguousarray(U[c * vs : (c + 1) * vs]),
                "sent": np.ascontiguousarray(
                    sentences[:, c * bs : (c + 1) * bs]
                ),
                "sent2": np.ascontiguousarray(
                    sent_shift[:, c * bs : (c + 1) * bs]
                ),
                "conv_w": conv_w,
                "conv_b": conv_b.reshape(1, h),
                "out_w": out_w,
                "out_b": out_b.reshape(1, 2),
                "patch": pt,
            }
        )
    return in_maps


def kernel(sentences, E, U, conv_w, conv_b, out_w, out_b):
    sentences = np.asarray(sentences, dtype=np.int32)
    E = np.asarray(E, dtype=np.float32)
    U = np.asarray(U, dtype=np.float32)
    conv_w = np.asarray(conv_w, dtype=np.float32)
    conv_b = np.asarray(conv_b, dtype=np.float32)
    out_w = np.asarray(out_w, dtype=np.float32)
    out_b = np.asarray(out_b, dtype=np.float32)

    nc = _get_nc()
    in_maps = make_in_maps(sentences, E, U, conv_w, conv_b, out_w, out_b)
    res = run_bass_kernel_spmd(nc, in_maps, list(range(NCORES)))
    return np.concatenate(
        [res.results[c]["scores"] for c in range(NCORES)], axis=0
    )



# revision 45
# speedup vs baseline: 6.9136x; 6.9136x over previous
"""Trainium Bass kernel for the ragged text-CNN problem.

Math: for tokens x[t,b] with embedding tables E,U [V,D] and conv weights
w [H, 2D, 2] (kernel width 2 over time):

    conv[b,h,t] = w0_h . e_{t,b} + w1_h . e_{t+1,b} + cb_h
    scores      = (max over valid t of conv) @ out_w.T + out_b

where e = concat(E[tok], U[tok]).  Since conv is linear in e, precompute a
fused per-token table T[v] = [e_v . w0 | e_v . w1] (128 wide), so
conv[b,:,t] = T[tok_t, 0:64] + T[tok_{t+1}, 64:128].  The ragged mask is
free: PAD (=1) appears exactly at positions t >= len, so forcing
T[PAD, 0:64] = -1e30 makes every masked conv position -1e30.

Distribution over 8 cores: pure batch-parallel, NO collectives.  Each core
only builds T rows for the ~7.6k distinct tokens in its own 32 sentences
(host computes the unique list + int16 remapped indices; all heavy data
movement and math stays on device):

  phase A: dma_gather (transposed) the unique concat(E,U) rows from the
           bf16-packed table in HBM -> SBUF in [feature, token] layout
  phase B: 128x128 matmuls project them to T -> DRAM (token-row layout)
  phase C: transposed dma_gather of T rows per token position
           -> [128 feat, token-cols]; conv = shifted add
  phase D: blocked tournament max over each sentence's 512 columns + bias
  phase E: tiny [65x32]@[65x2] head matmul
"""

import numpy as np
import ml_dtypes

try:
    import concourse.bass as bass
except ImportError:  # harness runs from a bare directory
    import sys

    sys.path.insert(0, "/opt/trn_rl_repo")
    import concourse.bass as bass

import concourse.mybir as mybir
from concourse.bacc import Bacc
import concourse.tile as tile
from concourse.bass_utils import run_bass_kernel_spmd

V, D, H, S, B = 50000, 300, 64, 512, 256
NCORES = 8
BS = B // NCORES          # sentences per core (32)
F = 2 * H                 # fused feature width (128)
P = 128
EW = 640                  # padded concat(E,U) width; bf16 row = 1280B (%256)
SPLIT = 32768             # int16 idx split for the vocab-row gather
NEG = -1.0e30
NCOLS = S * BS            # conv columns per core (16384)
CH = 256                  # phase A gather/matmul chunk (tokens)
CG = 2048                 # phase C gather chunk (token positions)

F32 = mybir.dt.float32
BF16 = mybir.dt.bfloat16
I16 = mybir.dt.int16

BF = ml_dtypes.bfloat16

# engine assignment ("v"=DVE, "g"=Pool) for the 8 conv adds / 4 group trees
ADD_ENG = "vvvvvvvv"
TREE_ENG = "vvvv"
CP_ENG = "aaaaaaaa"  # engine for the B-half staging copies ("a"/"v"/"g")


def build_nc(nu_lo, nu_hi, ws, hs, debug_probe=False):
    """Per-core SPMD program.

    nu_lo/nu_hi: padded unique-token counts (below/above the int16 split).
    ws: 4 group widths (cols per sentence for each 8-sentence length-sorted
    group), each a multiple of 128, <= 512.
    hs: for each group g, the number of hi-side phase-A chunks whose T rows
    cover every token the group's sentences use (the unique table is ordered
    by group first-use, so group g only reads t_dram[0 : nu_lo + hs[g]*CH]).
    """
    nu = nu_lo + nu_hi
    assert nu_lo % CH == 0 and nu_hi % CH == 0
    assert all(w % 128 == 0 and 128 <= w <= 512 for w in ws) and len(ws) == 4
    nhi = nu_hi // CH
    assert all(0 <= h <= nhi for h in hs) and list(hs) == sorted(hs)
    cw = [4 * ws[k // 2] for k in range(8)]      # cols per gather chunk
    ofs = np.concatenate([[0], np.cumsum(cw)])   # chunk col offsets
    CW = int(ofs[-1])                            # total conv cols

    nc = Bacc()
    eupad = nc.dram_tensor("eupad", [V, EW], BF16, kind="ExternalInput")
    w2 = nc.dram_tensor("w2", [P, 5 * P], BF16, kind="ExternalInput")
    eu_idx = nc.dram_tensor("eu_idx", [P, nu // 16], I16, kind="ExternalInput")
    ls_idx = nc.dram_tensor("ls_idx", [P, CW // 16], I16,
                            kind="ExternalInput")
    ow = nc.dram_tensor("ow", [H + 1, 2], F32, kind="ExternalInput")
    scores = nc.dram_tensor("scores", [BS, 2], F32, kind="ExternalOutput")
    t_dram = nc.dram_tensor("t_dram", [nu, P], BF16)
    if debug_probe:
        conv_out = nc.dram_tensor("conv_out", [H, CW], F32,
                                  kind="ExternalOutput")

    with tile.TileContext(nc) as tc:
        with (
            tc.tile_pool(name="c", bufs=1) as cpool,
            tc.tile_pool(name="g", bufs=3) as gpool,
            tc.tile_pool(name="ts", bufs=3) as tspool,
            tc.tile_pool(name="ga", bufs=3) as gapool,
            tc.tile_pool(name="tr", bufs=2) as trpool,
            tc.tile_pool(name="ps", bufs=3, space="PSUM") as psp,
        ):
            eu_idx_sb = cpool.tile([P, nu // 16], I16, tag="euidx")
            nc.sync.dma_start(eu_idx_sb[:], eu_idx[:, :])
            w2_sb = cpool.tile([P, 5 * P], BF16, tag="w2")
            nc.sync.dma_start(w2_sb[:], w2[:, :])
            w2v = w2_sb[:].rearrange("p (c j) -> p c j", c=5)
            ls_idx_sb = cpool.tile([P, CW // 16], I16, tag="lsidx")
            nc.sync.dma_start(ls_idx_sb[:], ls_idx[:, :])
            ow_sb = cpool.tile([H + 1, 2], F32, tag="ow")
            nc.sync.dma_start(ow_sb[:], ow[:, :])
            neg_sb = cpool.tile([1, H], BF16, tag="neg")
            nc.vector.memset(neg_sb[:], NEG)

            # ---- phase A/B helpers (gather unique EU rows + project to T)
            conv = cpool.tile([H, CW], BF16, tag="conv")
            pooled_t = cpool.tile([H + 1, BS], F32, tag="pooled")
            nc.vector.memset(pooled_t[H : H + 1, :], 1.0)

            a_ci = [0]

            def emit_a(tok0, lo):
                ci = a_ci[0]
                a_ci[0] += 1
                g = gpool.tile([P, 5, CH], BF16, tag="g", name="g")
                in_ap = eupad[0:SPLIT, :] if lo else eupad[SPLIT:V, :]
                nc.gpsimd.dma_gather(
                    out_ap=g[:, :, :],
                    in_ap=in_ap,
                    idxs_ap=eu_idx_sb[:, tok0 // 16 : (tok0 + CH) // 16],
                    num_idxs=CH,
                    num_idxs_reg=CH,
                    elem_size=EW,
                    transpose=True,
                    queue_num=0,
                    single_packet=False,
                )
                acc = psp.tile([P, CH], F32, tag="acc", name="acc")
                for sub in range(CH // P):
                    for c in range(5):
                        nc.tensor.matmul(
                            acc[:, sub * P : (sub + 1) * P],
                            lhsT=g[:, c, sub * P : (sub + 1) * P],
                            rhs=w2v[:, c, :],
                            start=(c == 0),
                            stop=(c == 4),
                        )
                ts = tspool.tile([P, CH], BF16, tag="ts", name="ts")
                if ci % 2 == 0:
                    nc.scalar.copy(ts[:, :], acc[:, :])
                else:
                    nc.vector.tensor_copy(ts[:, :], acc[:, :])
                nc.sync.dma_start(
                    t_dram[tok0 : tok0 + CH, :].rearrange(
                        "(s p) j -> p s j", p=P
                    ),
                    ts[:].rearrange("p (s j) -> p s j", j=P),
                )
                if ci == 0:
                    # PAD token is always local id 0: -inf its A half
                    # (the ragged mask for free); overlaps chunk 0's rows
                    # so it is ordered right after that write
                    nc.sync.dma_start(t_dram[0:1, 0:H], neg_sb[:, :])

            # ---- phase C/D helpers.  Sentences are length-sorted (desc) and
            # packed raggedly: chunk k holds 4 sentences of cw[k]/4 cols.
            add_eng = [nc.vector if ch == "v" else nc.gpsimd for ch in ADD_ENG]
            tree_eng = [nc.vector if ch == "v" else nc.gpsimd
                        for ch in TREE_ENG]

            class _Cp:
                def __init__(self, fn):
                    self.copy = fn

            cp_map = {
                "a": _Cp(nc.scalar.copy),
                "v": _Cp(nc.vector.tensor_copy),
            }
            cp_eng = [cp_map[ch] for ch in CP_ENG]
            gavs = {}

            def emit_c(k):
                i0 = int(ofs[k])
                n = cw[k]
                w = ws[k // 2]
                # sentences with len < w have -inf tail columns for free, so
                # the masking memset can run before the add (which rewrites
                # them with -1e30 + finite = -1e30); only the w=512 group can
                # have len == 512 sentences needing masking after the add
                if w < 512:
                    nc.vector.memset(conv[:, i0 + w - 1 : i0 + n : w], NEG)
                x = nu_lo + hs[k // 2] * CH
                ga = gapool.tile([P, 1, n], BF16, tag=f"ga{k}", bufs=1,
                                 name="ga")
                nc.gpsimd.dma_gather(
                    out_ap=ga[:, :, :],
                    in_ap=t_dram[0:x, :],
                    idxs_ap=ls_idx_sb[:, i0 // 16 : (i0 + n) // 16],
                    num_idxs=n,
                    num_idxs_reg=n,
                    elem_size=P,
                    transpose=True,
                    queue_num=0,
                    single_packet=False,
                )
                gavs[k] = ga[:].rearrange("p a b -> p (a b)")

            def emit_add(k):
                i0 = int(ofs[k])
                n = cw[k]
                w = ws[k // 2]
                gav = gavs[k]
                # conv[f, i] = TA[tok_i] + TB[tok_{i+1}]; stop one short of
                # the chunk's end (its last column is a per-sentence tail
                # column, masked in emit_c / below).  The BIR verifier
                # requires both tensor_tensor SBUF inputs at the same base
                # partition, so stage the shifted B half at base 0 with a
                # single-input copy first (partition-shifting copies are
                # legal).
                tb = gapool.tile([H, n - 1], BF16, tag=f"tb{k}", bufs=1,
                                 name="tb")
                cp_eng[k].copy(tb[:, :], gav[H:F, 1:n])
                add_eng[k].tensor_add(
                    conv[:, i0 : i0 + n - 1],
                    gav[0:H, 0 : n - 1],
                    tb[:, :],
                )
                if w == 512:
                    nc.vector.memset(conv[:, i0 + w - 1 : i0 + n : w], NEG)

            def emit_tree(grp):
                w = ws[grp]
                g0 = int(ofs[2 * grp])
                v = conv[:, g0 : g0 + 8 * w].rearrange(
                    "p (b t) -> p b t", t=w
                )
                eng = tree_eng[grp]
                cur = None
                w_ = w
                r = 0
                while w_ > 32:
                    nxt = trpool.tile([H, 8, w_ // 2], BF16,
                                      tag=f"tr{grp}_{r}", name="nxt")
                    src0 = (v[:, :, 0 : w_ // 2] if cur is None
                            else cur[:, :, 0 : w_ // 2])
                    src1 = (v[:, :, w_ // 2 : w_] if cur is None
                            else cur[:, :, w_ // 2 : w_])
                    eng.tensor_max(nxt[:, :, :], src0, src1)
                    cur = nxt
                    w_ //= 2
                    r += 1
                red = trpool.tile([H, 8, 1], F32, tag=f"red{grp}", name="red")
                nc.vector.reduce_max(
                    red[:, :, :], cur[:, :, :], axis=mybir.AxisListType.X
                )
                nc.vector.tensor_copy(
                    pooled_t[0:H, grp * 8 : (grp + 1) * 8],
                    red[:].rearrange("p a b -> p (a b)"),
                )

            # ---- emission schedule: lo chunks, then hi chunks with groups
            # 0/1's C gathers slotted in as soon as their T prefix is done
            # (with a ~3-gather lag so the t-write drain elapses first).
            # Pool-side adds are emitted right after the last phase-A gather
            # so they fill Pool's wait for the final t_dram writes instead
            # of stalling before C2.
            for o in range(0, nu_lo, CH):
                emit_a(o, True)
            pts = [min(nhi, hs[0] + 3)]
            pts.append(min(nhi, max(hs[1] + 3, pts[0] + 2)))
            hi_done = 0
            pool_adds = []
            trees_pending = []
            for g in range(2):
                while hi_done < pts[g]:
                    emit_a(nu_lo + hi_done * CH, False)
                    hi_done += 1
                emit_c(2 * g)
                emit_c(2 * g + 1)
                dve_only = True
                for k in (2 * g, 2 * g + 1):
                    if ADD_ENG[k] == "v":
                        emit_add(k)
                    else:
                        pool_adds.append(k)
                        dve_only = False
                if dve_only:
                    emit_tree(g)
                else:
                    trees_pending.append(g)
            while hi_done < nhi:
                emit_a(nu_lo + hi_done * CH, False)
                hi_done += 1
            for k in pool_adds:
                emit_add(k)
            for g in trees_pending:
                emit_tree(g)
            emit_c(4)
            emit_c(5)
            emit_add(4)
            emit_add(5)
            emit_tree(2)
            emit_c(6)
            emit_c(7)
            emit_add(6)
            emit_add(7)
            emit_tree(3)

            if debug_probe:
                conv_f = cpool.tile([H, CW], F32, tag="conv_f")
                nc.vector.tensor_copy(conv_f[:], conv[:])
                nc.sync.dma_start(conv_out[:, :], conv_f[:])

            # ---- phase E: head
            sc_ps = psp.tile([BS, 2], F32, tag="sc")
            nc.tensor.matmul(
                sc_ps[:, :],
                lhsT=pooled_t[:, :],
                rhs=ow_sb[:, :],
                start=True,
                stop=True,
            )
            sc_sb = cpool.tile([BS, 2], F32, tag="scsb")
            nc.vector.tensor_copy(sc_sb[:], sc_ps[:])
            nc.sync.dma_start(scores[:, :], sc_sb[:])

    nc.finalize()
    return nc


_NC_CACHE = {}


def _get_nc(nu_lo=None, nu_hi=None, ws=None, hs=None):
    if nu_lo is None:
        return _NC_CACHE["last"]
    key = (nu_lo, nu_hi, ws, hs)
    if key not in _NC_CACHE:
        _NC_CACHE[key] = build_nc(nu_lo, nu_hi, ws, hs)
    _NC_CACHE["last"] = _NC_CACHE[key]
    return _NC_CACHE[key]


def _wrap16(a):
    """Pack idx stream into the swdge wrap layout: idx i at
    [partition i%16, col i//16], replicated across all 16-row groups."""
    n = len(a)
    w = a.reshape(n // 16, 16).T
    out = np.zeros((P, n // 16), np.int16)
    for g in range(8):
        out[16 * g : 16 * (g + 1)] = w
    return out


def _round_up(x, m):
    return (x + m - 1) // m * m


def make_in_maps(sentences, E, U, conv_w, conv_b, out_w, out_b):
    sentences = np.asarray(sentences, np.int32)
    eupad = np.zeros((V, EW), BF)
    eupad[:, 0:D] = np.asarray(E, np.float32).astype(BF)
    eupad[:, D : 2 * D] = np.asarray(U, np.float32).astype(BF)

    w2full = np.zeros((EW, F), np.float32)
    w2full[0 : 2 * D, 0:H] = np.asarray(conv_w, np.float32)[:, :, 0].T
    w2full[0 : 2 * D, H:F] = np.asarray(conv_w, np.float32)[:, :, 1].T
    w2sb = np.zeros((P, 5 * P), np.float32)
    for c in range(5):
        w2sb[:, c * P : (c + 1) * P] = w2full[c * P : (c + 1) * P, :]
    w2sb = w2sb.astype(BF)

    # fold the conv bias into the head bias row:
    # (pooled + cb) @ ow^T + ob == pooled @ ow^T + (ob + ow @ cb)
    ow_f = np.asarray(out_w, np.float32)
    bias2 = np.asarray(out_b, np.float32) + ow_f @ np.asarray(
        conv_b, np.float32
    )
    owh = np.concatenate([ow_f.T, bias2.reshape(1, 2)], axis=0)

    # Balance work across cores: greedily assign sentences (longest first)
    # to the least-loaded core.  This roughly equalizes per-core distinct
    # token counts, shrinking the padded unique-table size (the SPMD
    # program must be sized for the max across cores).
    lens = (sentences != 1).sum(axis=0)
    order = np.argsort(-lens, kind="stable")
    load = np.zeros(NCORES, np.int64)
    assign = [[] for _ in range(NCORES)]
    for b in order:
        open_cores = [c for c in range(NCORES) if len(assign[c]) < BS]
        c = min(open_cores, key=lambda c: load[c])
        assign[c].append(int(b))
        load[c] += int(lens[b]) + 1
    perm = np.array([b for c in range(NCORES) for b in assign[c]], np.int64)

    # Ragged group widths: each core's sentences are length-sorted (the
    # greedy above appends in global desc order), split into 4 groups of 8.
    # Group width = max(len)+1 in the group across cores, rounded to 128.
    ws = tuple(
        min(512, _round_up(int(max(lens[assign[c][8 * g]]
                                   for c in range(NCORES))) + 1, 128))
        for g in range(4)
    )

    # Unique token lists in GROUP-FIRST-USE order (PAD first), so that each
    # phase-C group only depends on a prefix of the T table.
    per_core = []
    r_hi_all = []
    for c in range(NCORES):
        toks = sentences[:, assign[c]]
        seen = np.zeros(V, bool)
        seen[1] = True
        lo_parts = [np.array([1], np.int64)]
        hi_parts = []
        r_hi = []
        for g in range(4):
            su = np.concatenate(
                [toks[0 : ws[g], r] for r in range(8 * g, 8 * (g + 1))]
            )
            u, fi = np.unique(su, return_index=True)
            order = u[np.argsort(fi)]
            new = order[~seen[order]]
            seen[new] = True
            lo_parts.append(new[new < SPLIT])
            hi_parts.append(new[new >= SPLIT])
            r_hi.append(sum(len(h) for h in hi_parts))
        lo = np.concatenate(lo_parts)
        hi = np.concatenate(hi_parts)
        per_core.append((toks, lo, hi))
        r_hi_all.append(r_hi)

    nu_lo = _round_up(max(len(lo) for _, lo, _ in per_core), CH)
    nu_hi = _round_up(max(1, max(len(hi) for _, _, hi in per_core)), CH)
    nhi = nu_hi // CH
    hs = tuple(
        min(nhi, -(-max(r_hi_all[c][g] for c in range(NCORES)) // CH))
        for g in range(4)
    )

    in_maps = []
    for toks, lo, hi in per_core:
        eu = np.zeros(nu_lo + nu_hi, np.int16)
        eu[0 : len(lo)] = lo.astype(np.int16)
        eu[nu_lo : nu_lo + len(hi)] = (hi - SPLIT).astype(np.int16)
        lut = np.zeros(V, np.int32)
        lut[lo] = np.arange(len(lo))
        lut[hi] = nu_lo + np.arange(len(hi))
        lsl = lut[toks]  # [S, BS] local ids, column r = rank-r sentence
        ls = np.concatenate(
            [lsl[0 : ws[r // 8], r] for r in range(BS)]
        ).astype(np.int16)
        in_maps.append(
            {
                "eupad": eupad,
                "w2": w2sb,
                "eu_idx": _wrap16(eu),
                "ls_idx": _wrap16(ls),
                "ow": owh,
            }
        )
    return nu_lo, nu_hi, ws, hs, in_maps, perm


def kernel(sentences, E, U, conv_w, conv_b, out_w, out_b):
    nu_lo, nu_hi, ws, hs, in_maps, perm = make_in_maps(
        sentences, E, U, conv_w, conv_b, out_w, out_b
    )
    nc = _get_nc(nu_lo, nu_hi, ws, hs)
    res = run_bass_kernel_spmd(nc, in_maps, list(range(NCORES)))
    packed = np.concatenate(
        [res.results[c]["scores"] for c in range(NCORES)], axis=0
    )
    out = np.empty_like(packed)
    out[perm] = packed
    return out


# revision 49
# speedup vs baseline: 7.1064x; 1.0279x over previous
"""Trainium Bass kernel for the ragged text-CNN problem.

Math: for tokens x[t,b] with embedding tables E,U [V,D] and conv weights
w [H, 2D, 2] (kernel width 2 over time):

    conv[b,h,t] = w0_h . e_{t,b} + w1_h . e_{t+1,b} + cb_h
    scores      = (max over valid t of conv) @ out_w.T + out_b

where e = concat(E[tok], U[tok]).  Since conv is linear in e, precompute a
fused per-token table T[v] = [e_v . w0 | e_v . w1] (128 wide), so
conv[b,:,t] = T[tok_t, 0:64] + T[tok_{t+1}, 64:128].  The ragged mask is
free: PAD (=1) appears exactly at positions t >= len, so forcing
T[PAD, 0:64] = -1e30 makes every masked conv position -1e30.

Distribution over 8 cores: pure batch-parallel, NO collectives.  Each core
only builds T rows for the ~7.6k distinct tokens in its own 32 sentences
(host computes the unique list + int16 remapped indices; all heavy data
movement and math stays on device):

  phase A: dma_gather (transposed) the unique concat(E,U) rows from the
           bf16-packed table in HBM -> SBUF in [feature, token] layout
  phase B: 128x128 matmuls project them to T -> DRAM (token-row layout)
  phase C: transposed dma_gather of T rows per token position
           -> [128 feat, token-cols]; conv = shifted add
  phase D: blocked tournament max over each sentence's 512 columns + bias
  phase E: tiny [65x32]@[65x2] head matmul
"""

import numpy as np
import ml_dtypes

try:
    import concourse.bass as bass
except ImportError:  # harness runs from a bare directory
    import sys

    sys.path.insert(0, "/opt/trn_rl_repo")
    import concourse.bass as bass

import concourse.mybir as mybir
from concourse.bacc import Bacc
import concourse.tile as tile
from concourse.bass_utils import run_bass_kernel_spmd

V, D, H, S, B = 50000, 300, 64, 512, 256
NCORES = 8
BS = B // NCORES          # sentences per core (32)
F = 2 * H                 # fused feature width (128)
P = 128
EW = 640                  # padded concat(E,U) width; bf16 row = 1280B (%256)
SPLIT = 32768             # int16 idx split for the vocab-row gather
NEG = -1.0e30
NCOLS = S * BS            # conv columns per core (16384)
CH = 256                  # phase A gather/matmul chunk (tokens)
CG = 2048                 # phase C gather chunk (token positions)

F32 = mybir.dt.float32
BF16 = mybir.dt.bfloat16
I16 = mybir.dt.int16

BF = ml_dtypes.bfloat16

# engine assignment ("v"=DVE, "g"=Pool) for the 8 conv adds / 4 group trees
ADD_ENG = "vvvvvvvv"
TREE_ENG = "vvvv"
CP_ENG = "aaaaaaaa"  # engine for the B-half staging copies ("a"/"v"/"g")


def build_nc(nu_lo, nu_hi, ws, hs, debug_probe=False):
    """Per-core SPMD program.

    nu_lo/nu_hi: padded unique-token counts (below/above the int16 split).
    ws: 4 group widths (cols per sentence for each 8-sentence length-sorted
    group), each a multiple of 128, <= 512.
    hs: for each group g, the number of hi-side phase-A chunks whose T rows
    cover every token the group's sentences use (the unique table is ordered
    by group first-use, so group g only reads t_dram[0 : nu_lo + hs[g]*CH]).
    """
    nu = nu_lo + nu_hi
    assert nu_lo % CH == 0 and nu_hi % CH == 0
    assert all(w % 128 == 0 and 128 <= w <= 512 for w in ws) and len(ws) == 4
    nhi = nu_hi // CH
    assert all(0 <= h <= nhi for h in hs) and list(hs) == sorted(hs)
    cw = [4 * ws[k // 2] for k in range(8)]      # cols per gather chunk
    ofs = np.concatenate([[0], np.cumsum(cw)])   # chunk col offsets
    CW = int(ofs[-1])                            # total conv cols
    GW = CW + 8 * P                              # gathered cols (+pad/chunk)

    nc = Bacc()
    eupad = nc.dram_tensor("eupad", [V, EW], BF16, kind="ExternalInput")
    w2 = nc.dram_tensor("w2", [P, 5 * P], BF16, kind="ExternalInput")
    eu_idx = nc.dram_tensor("eu_idx", [P, nu // 16], I16, kind="ExternalInput")
    ls_idx = nc.dram_tensor("ls_idx", [P, GW // 16], I16,
                            kind="ExternalInput")
    ow = nc.dram_tensor("ow", [H + 1, 2], F32, kind="ExternalInput")
    sel = nc.dram_tensor("sel", [P, P], BF16, kind="ExternalInput")
    scores = nc.dram_tensor("scores", [BS, 2], F32, kind="ExternalOutput")
    t_dram = nc.dram_tensor("t_dram", [nu, P], BF16)
    if debug_probe:
        conv_out = nc.dram_tensor("conv_out", [H, CW], F32,
                                  kind="ExternalOutput")

    with tile.TileContext(nc) as tc:
        with (
            tc.tile_pool(name="c", bufs=1) as cpool,
            tc.tile_pool(name="g", bufs=3) as gpool,
            tc.tile_pool(name="ts", bufs=3) as tspool,
            tc.tile_pool(name="ga", bufs=3) as gapool,
            tc.tile_pool(name="tr", bufs=2) as trpool,
            tc.tile_pool(name="ps", bufs=3, space="PSUM") as psp,
        ):
            eu_idx_sb = cpool.tile([P, nu // 16], I16, tag="euidx")
            nc.sync.dma_start(eu_idx_sb[:], eu_idx[:, :])
            w2_sb = cpool.tile([P, 5 * P], BF16, tag="w2")
            nc.sync.dma_start(w2_sb[:], w2[:, :])
            w2v = w2_sb[:].rearrange("p (c j) -> p c j", c=5)
            ls_idx_sb = cpool.tile([P, GW // 16], I16, tag="lsidx")
            nc.sync.dma_start(ls_idx_sb[:], ls_idx[:, :])
            ow_sb = cpool.tile([H + 1, 2], F32, tag="ow")
            nc.sync.dma_start(ow_sb[:], ow[:, :])
            sel_sb = cpool.tile([P, P], BF16, tag="sel")
            nc.sync.dma_start(sel_sb[:], sel[:, :])
            neg_sb = cpool.tile([1, H], BF16, tag="neg")
            nc.vector.memset(neg_sb[:], NEG)

            # ---- phase A/B helpers (gather unique EU rows + project to T)
            conv = cpool.tile([H, CW], BF16, tag="conv")
            pooled_t = cpool.tile([H + 1, BS], F32, tag="pooled")
            nc.vector.memset(pooled_t[H : H + 1, :], 1.0)

            a_ci = [0]

            def emit_a(tok0, lo):
                ci = a_ci[0]
                a_ci[0] += 1
                g = gpool.tile([P, 5, CH], BF16, tag="g", name="g")
                in_ap = eupad[0:SPLIT, :] if lo else eupad[SPLIT:V, :]
                nc.gpsimd.dma_gather(
                    out_ap=g[:, :, :],
                    in_ap=in_ap,
                    idxs_ap=eu_idx_sb[:, tok0 // 16 : (tok0 + CH) // 16],
                    num_idxs=CH,
                    num_idxs_reg=CH,
                    elem_size=EW,
                    transpose=True,
                    queue_num=0,
                    single_packet=False,
                )
                acc = psp.tile([P, CH], F32, tag="acc", name="acc", bufs=2)
                for sub in range(CH // P):
                    for c in range(5):
                        nc.tensor.matmul(
                            acc[:, sub * P : (sub + 1) * P],
                            lhsT=g[:, c, sub * P : (sub + 1) * P],
                            rhs=w2v[:, c, :],
                            start=(c == 0),
                            stop=(c == 4),
                        )
                ts = tspool.tile([P, CH], BF16, tag="ts", name="ts")
                if ci % 2 == 0:
                    nc.scalar.copy(ts[:, :], acc[:, :])
                else:
                    nc.vector.tensor_copy(ts[:, :], acc[:, :])
                nc.sync.dma_start(
                    t_dram[tok0 : tok0 + CH, :].rearrange(
                        "(s p) j -> p s j", p=P
                    ),
                    ts[:].rearrange("p (s j) -> p s j", j=P),
                )
                if ci == 0:
                    # PAD token is always local id 0: -inf its A half
                    # (the ragged mask for free); overlaps chunk 0's rows
                    # so it is ordered right after that write
                    nc.sync.dma_start(t_dram[0:1, 0:H], neg_sb[:, :])

            # ---- phase C/D helpers.  Sentences are length-sorted (desc) and
            # packed raggedly: chunk k holds 4 sentences of cw[k]/4 cols.
            add_eng = [nc.vector if ch == "v" else nc.gpsimd for ch in ADD_ENG]
            tree_eng = [nc.vector if ch == "v" else nc.gpsimd
                        for ch in TREE_ENG]

            class _Cp:
                def __init__(self, fn):
                    self.copy = fn

            cp_map = {
                "a": _Cp(nc.scalar.copy),
                "v": _Cp(nc.vector.tensor_copy),
            }
            cp_eng = [cp_map[ch] for ch in CP_ENG]
            gavs = {}

            def emit_c(k):
                i0 = int(ofs[k])
                n = cw[k]
                w = ws[k // 2]
                # sentences with len < w have -inf tail columns for free, so
                # the masking memset can run before the add (which rewrites
                # them with -1e30 + finite = -1e30); only the w=512 group can
                # have len == 512 sentences needing masking after the add
                if w < 512:
                    nc.vector.memset(conv[:, i0 + w - 1 : i0 + n : w], NEG)
                x = nu_lo + hs[k // 2] * CH
                gi0 = i0 + k * P   # idx-stream offset (128 pad idx/chunk)
                ga = gapool.tile([P, 1, n + P], BF16, tag=f"ga{k}", bufs=1,
                                 name="ga")
                nc.gpsimd.dma_gather(
                    out_ap=ga[:, :, :],
                    in_ap=t_dram[0:x, :],
                    idxs_ap=ls_idx_sb[:, gi0 // 16 : (gi0 + n + P) // 16],
                    num_idxs=n + P,
                    num_idxs_reg=n + P,
                    elem_size=P,
                    transpose=True,
                    queue_num=0,
                    single_packet=False,
                )
                gavs[k] = ga[:].rearrange("p a b -> p (a b)")

            def emit_add(k):
                # conv[f, i] = TA[tok_i] + TB[tok_{i+1}] via two accumulating
                # PE matmuls with 0/1 selection matrices (tensor_tensor with
                # mismatched SBUF base partitions is illegal, and elementwise
                # ops only exist on DVE -- PE is idle here).  Emitted per
                # half-chunk to bound PSUM tile size; the conv tile is then
                # staged to SBUF bf16 by the Activation engine.  The gather
                # has 128 pad columns per chunk so the shifted rhs never
                # leaves the tile.
                i0 = int(ofs[k])
                n = cw[k]
                w = ws[k // 2]
                gav = gavs[k]
                for h0 in (0, n // 2):
                    hn = n // 2
                    cps = psp.tile([H, 1024], F32, tag="cps", bufs=2,
                                   name="cps")
                    for p0 in range(0, hn, 512):
                        pn = min(512, hn - p0)
                        gp = h0 + p0
                        nc.tensor.matmul(
                            cps[:, p0 : p0 + pn],
                            lhsT=sel_sb[:, 0:H],
                            rhs=gav[:, gp : gp + pn],
                            start=True, stop=False)
                        nc.tensor.matmul(
                            cps[:, p0 : p0 + pn],
                            lhsT=sel_sb[:, H:F],
                            rhs=gav[:, gp + 1 : gp + pn + 1],
                            start=False, stop=True)
                    nc.scalar.copy(conv[:, i0 + h0 : i0 + h0 + hn],
                                   cps[:, 0:hn])
                if w == 512:
                    nc.vector.memset(conv[:, i0 + w - 1 : i0 + n : w], NEG)

            def emit_tree(grp):
                w = ws[grp]
                g0 = int(ofs[2 * grp])
                v = conv[:, g0 : g0 + 8 * w].rearrange(
                    "p (b t) -> p b t", t=w
                )
                eng = tree_eng[grp]
                cur = None
                w_ = w
                r = 0
                while w_ > 32:
                    nxt = trpool.tile([H, 8, w_ // 2], BF16,
                                      tag=f"tr{grp}_{r}", name="nxt")
                    src0 = (v[:, :, 0 : w_ // 2] if cur is None
                            else cur[:, :, 0 : w_ // 2])
                    src1 = (v[:, :, w_ // 2 : w_] if cur is None
                            else cur[:, :, w_ // 2 : w_])
                    eng.tensor_max(nxt[:, :, :], src0, src1)
                    cur = nxt
                    w_ //= 2
                    r += 1
                red = trpool.tile([H, 8, 1], F32, tag=f"red{grp}", name="red")
                nc.vector.reduce_max(
                    red[:, :, :], cur[:, :, :], axis=mybir.AxisListType.X
                )
                nc.vector.tensor_copy(
                    pooled_t[0:H, grp * 8 : (grp + 1) * 8],
                    red[:].rearrange("p a b -> p (a b)"),
                )

            # ---- emission schedule: lo chunks, then hi chunks with groups
            # 0/1's C gathers slotted in as soon as their T prefix is done
            # (with a ~3-gather lag so the t-write drain elapses first).
            # Pool-side adds are emitted right after the last phase-A gather
            # so they fill Pool's wait for the final t_dram writes instead
            # of stalling before C2.
            for o in range(0, nu_lo, CH):
                emit_a(o, True)
            pts = [min(nhi, hs[0] + 3)]
            pts.append(min(nhi, max(hs[1] + 3, pts[0] + 2)))
            hi_done = 0
            pool_adds = []
            trees_pending = []
            for g in range(2):
                while hi_done < pts[g]:
                    emit_a(nu_lo + hi_done * CH, False)
                    hi_done += 1
                emit_c(2 * g)
                emit_c(2 * g + 1)
                dve_only = True
                for k in (2 * g, 2 * g + 1):
                    if ADD_ENG[k] == "v":
                        emit_add(k)
                    else:
                        pool_adds.append(k)
                        dve_only = False
                if dve_only:
                    emit_tree(g)
                else:
                    trees_pending.append(g)
            while hi_done < nhi:
                emit_a(nu_lo + hi_done * CH, False)
                hi_done += 1
            for k in pool_adds:
                emit_add(k)
            for g in trees_pending:
                emit_tree(g)
            emit_c(4)
            emit_c(5)
            emit_add(4)
            emit_add(5)
            emit_tree(2)
            emit_c(6)
            emit_c(7)
            emit_add(6)
            emit_add(7)
            emit_tree(3)

            if debug_probe:
                conv_f = cpool.tile([H, CW], F32, tag="conv_f")
                nc.vector.tensor_copy(conv_f[:], conv[:])
                nc.sync.dma_start(conv_out[:, :], conv_f[:])

            # ---- phase E: head
            sc_ps = psp.tile([BS, 2], F32, tag="sc", bufs=1)
            nc.tensor.matmul(
                sc_ps[:, :],
                lhsT=pooled_t[:, :],
                rhs=ow_sb[:, :],
                start=True,
                stop=True,
            )
            sc_sb = cpool.tile([BS, 2], F32, tag="scsb")
            nc.vector.tensor_copy(sc_sb[:], sc_ps[:])
            nc.sync.dma_start(scores[:, :], sc_sb[:])

    nc.finalize()
    return nc


_NC_CACHE = {}


def _get_nc(nu_lo=None, nu_hi=None, ws=None, hs=None):
    if nu_lo is None:
        return _NC_CACHE["last"]
    key = (nu_lo, nu_hi, ws, hs)
    if key not in _NC_CACHE:
        _NC_CACHE[key] = build_nc(nu_lo, nu_hi, ws, hs)
    _NC_CACHE["last"] = _NC_CACHE[key]
    return _NC_CACHE[key]


def _wrap16(a):
    """Pack idx stream into the swdge wrap layout: idx i at
    [partition i%16, col i//16], replicated across all 16-row groups."""
    n = len(a)
    w = a.reshape(n // 16, 16).T
    out = np.zeros((P, n // 16), np.int16)
    for g in range(8):
        out[16 * g : 16 * (g + 1)] = w
    return out


def _round_up(x, m):
    return (x + m - 1) // m * m


def make_in_maps(sentences, E, U, conv_w, conv_b, out_w, out_b):
    sentences = np.asarray(sentences, np.int32)
    eupad = np.zeros((V, EW), BF)
    eupad[:, 0:D] = np.asarray(E, np.float32).astype(BF)
    eupad[:, D : 2 * D] = np.asarray(U, np.float32).astype(BF)

    w2full = np.zeros((EW, F), np.float32)
    w2full[0 : 2 * D, 0:H] = np.asarray(conv_w, np.float32)[:, :, 0].T
    w2full[0 : 2 * D, H:F] = np.asarray(conv_w, np.float32)[:, :, 1].T
    w2sb = np.zeros((P, 5 * P), np.float32)
    for c in range(5):
        w2sb[:, c * P : (c + 1) * P] = w2full[c * P : (c + 1) * P, :]
    w2sb = w2sb.astype(BF)

    # fold the conv bias into the head bias row:
    # (pooled + cb) @ ow^T + ob == pooled @ ow^T + (ob + ow @ cb)
    ow_f = np.asarray(out_w, np.float32)
    bias2 = np.asarray(out_b, np.float32) + ow_f @ np.asarray(
        conv_b, np.float32
    )
    owh = np.concatenate([ow_f.T, bias2.reshape(1, 2)], axis=0)
    selh = np.zeros((P, P), np.float32)
    selh[np.arange(H), np.arange(H)] = 1.0            # SEL_A: k == f
    selh[H + np.arange(H), H + np.arange(H)] = 1.0    # SEL_B: k == 64+f
    selh = selh.astype(BF)

    # Balance work across cores: greedily assign sentences (longest first)
    # to the least-loaded core.  This roughly equalizes per-core distinct
    # token counts, shrinking the padded unique-table size (the SPMD
    # program must be sized for the max across cores).
    lens = (sentences != 1).sum(axis=0)
    order = np.argsort(-lens, kind="stable")
    load = np.zeros(NCORES, np.int64)
    assign = [[] for _ in range(NCORES)]
    for b in order:
        open_cores = [c for c in range(NCORES) if len(assign[c]) < BS]
        c = min(open_cores, key=lambda c: load[c])
        assign[c].append(int(b))
        load[c] += int(lens[b]) + 1
    perm = np.array([b for c in range(NCORES) for b in assign[c]], np.int64)

    # Ragged group widths: each core's sentences are length-sorted (the
    # greedy above appends in global desc order), split into 4 groups of 8.
    # Group width = max(len)+1 in the group across cores, rounded to 128.
    ws = tuple(
        min(512, _round_up(int(max(lens[assign[c][8 * g]]
                                   for c in range(NCORES))) + 1, 128))
        for g in range(4)
    )

    # Unique token lists in GROUP-FIRST-USE order (PAD first), so that each
    # phase-C group only depends on a prefix of the T table.
    per_core = []
    r_hi_all = []
    for c in range(NCORES):
        toks = sentences[:, assign[c]]
        seen = np.zeros(V, bool)
        seen[1] = True
        lo_parts = [np.array([1], np.int64)]
        hi_parts = []
        r_hi = []
        for g in range(4):
            su = np.concatenate(
                [toks[0 : ws[g], r] for r in range(8 * g, 8 * (g + 1))]
            )
            u, fi = np.unique(su, return_index=True)
            order = u[np.argsort(fi)]
            new = order[~seen[order]]
            seen[new] = True
            lo_parts.append(new[new < SPLIT])
            hi_parts.append(new[new >= SPLIT])
            r_hi.append(sum(len(h) for h in hi_parts))
        lo = np.concatenate(lo_parts)
        hi = np.concatenate(hi_parts)
        per_core.append((toks, lo, hi))
        r_hi_all.append(r_hi)

    nu_lo = _round_up(max(len(lo) for _, lo, _ in per_core), CH)
    nu_hi = _round_up(max(1, max(len(hi) for _, _, hi in per_core)), CH)
    nhi = nu_hi // CH
    hs = tuple(
        min(nhi, -(-max(r_hi_all[c][g] for c in range(NCORES)) // CH))
        for g in range(4)
    )

    in_maps = []
    for toks, lo, hi in per_core:
        eu = np.zeros(nu_lo + nu_hi, np.int16)
        eu[0 : len(lo)] = lo.astype(np.int16)
        eu[nu_lo : nu_lo + len(hi)] = (hi - SPLIT).astype(np.int16)
        lut = np.zeros(V, np.int32)
        lut[lo] = np.arange(len(lo))
        lut[hi] = nu_lo + np.arange(len(hi))
        lsl = lut[toks]  # [S, BS] local ids, column r = rank-r sentence
        parts = []
        for k in range(8):
            for r in range(4 * k, 4 * (k + 1)):
                parts.append(lsl[0 : ws[r // 8], r])
            parts.append(np.zeros(P, np.int64))  # pad idx per chunk
        ls = np.concatenate(parts).astype(np.int16)
        in_maps.append(
            {
                "eupad": eupad,
                "w2": w2sb,
                "eu_idx": _wrap16(eu),
                "ls_idx": _wrap16(ls),
                "ow": owh,
                "sel": selh,
            }
        )
    return nu_lo, nu_hi, ws, hs, in_maps, perm


def kernel(sentences, E, U, conv_w, conv_b, out_w, out_b):
    nu_lo, nu_hi, ws, hs, in_maps, perm = make_in_maps(
        sentences, E, U, conv_w, conv_b, out_w, out_b
    )
    nc = _get_nc(nu_lo, nu_hi, ws, hs)
    res = run_bass_kernel_spmd(nc, in_maps, list(range(NCORES)))
    packed = np.concatenate(
        [res.results[c]["scores"] for c in range(NCORES)], axis=0
    )
    out = np.empty_like(packed)
    out[perm] = packed
    return out
